# revision 38
# baseline (speedup 1.0000x reference)
"""Trainium2 Bass kernel for nn_DUDCLoss_1382979469646.

Data-parallel over the batch dim: 8 cores x 512 rows each. The loss is
factorized so each row needs only a handful of C-length passes, and the
eps=1e-5 inside log(q+eps) is dropped (rel err ~1.3e-3, tolerance 2e-2).

The device computes, per row, only the six C-length reductions
  E1 = sum exp(x1), E2 = sum exp(x2),
  G12 = sum exp(x1)*x2, G21 = sum exp(x2)*x1,
  M12 = sum sigmoid(x1)*logsigmoid(x2), M21 (accumulated as
        sum (r2-1)*u1 = -M21 via r2 = 1/(1+exp(x2)))
and exports them as a [128, 6T] tile. The host (which already holds the
gathered positive logits g) finishes the tiny [B,K] part in fp64.

M12 goes through s1 = exp(u1) on ACT for three tiles; one MID tile
(FOLD) instead folds M12 through r1 = 1/(1+exp(x1)) on DVE, so the
extra DVE work lands in DVE's input-starved mid-stream gaps and the
kernel tail is the short chain sg -> pm -> reduce of the last tile.

Engine balance per [128, 1024]-pair tile (ns):
  ACT : exp 1892, ln(1+A) 1892, s1=exp(u1) 1038 (SG tiles)
  Pool: u = x - ln(1+A) 1707, bf16 product mults 853 each
  DVE : E reduces via 4x-mode tensor_scalar+accum (327 each, B = 1+A
        folded in via op0=add), r=recip(B) 1127, G/M12 reduces 327,
        fused M-stt (r-1)*u 1127
"""

import numpy as np

NCORES = 8
B, C, K = 4096, 1024, 8
RPC = B // NCORES          # rows per core
P = 128                    # partitions
T = RPC // P               # row-tiles per core
TK = T * K
EPS = 1e-5
FOLD = 1                   # tile whose M12 is folded on DVE
NOUT = 6 * T

_cache = {}


def _patch_act_tables(mybir, bacc):
    """Make the ACT-table-load inserter resolve both Exp and Ln to the one
    set that holds both (natural_log_exp_and_others). The default policy
    picks a singleton set per function, inserting a ~1.3us table load at
    every Exp<->Ln transition in the scheduled stream."""
    if getattr(bacc, "_dudc_act_patch", False):
        return
    orig = bacc.get_activation_tables
    both = {mybir.ActivationFunctionType.Exp, mybir.ActivationFunctionType.Ln}

    def patched(arch):
        tabs = orig(arch)
        if any(both <= funcs for funcs in tabs.values()):
            for name, funcs in tabs.items():
                if not both <= funcs:
                    funcs.difference_update(both)
        return tabs

    bacc.get_activation_tables = patched
    bacc._dudc_act_patch = True


def _build():
    import concourse.bass as bass
    import concourse.tile as tile
    from concourse import bacc, mybir

    _patch_act_tables(mybir, bacc)

    fp32 = mybir.dt.float32
    bf16 = mybir.dt.bfloat16
    AF = mybir.ActivationFunctionType
    ALU = mybir.AluOpType

    nc = bacc.Bacc(
        "TRN2",
        target_bir_lowering=False,
        debug=False,
        num_devices=NCORES,
    )

    x1d = nc.dram_tensor("x1", [RPC, C], fp32, kind="ExternalInput").ap()
    x2d = nc.dram_tensor("x2", [RPC, C], fp32, kind="ExternalInput").ap()
    outd = nc.dram_tensor("out", [P, NOUT], fp32, kind="ExternalOutput").ap()

    H = C // 2
    LAST = T - 1

    with tile.TileContext(nc) as tc:
        with (
            tc.tile_pool(name="x", bufs=T) as xp,
            tc.tile_pool(name="A", bufs=2) as ap_,
            tc.tile_pool(name="llp", bufs=2) as llpp,
            tc.tile_pool(name="u", bufs=T) as up,
            tc.tile_pool(name="br", bufs=2) as brp,
            tc.tile_pool(name="sg", bufs=3) as sgp,
            tc.tile_pool(name="pr", bufs=3) as prp,
            tc.tile_pool(name="small", bufs=1) as sm,
        ):
            # out columns: [E1+C | E2+C | G12 | G21 | M12 | M21neg] x T
            outt = sm.tile([P, NOUT], fp32)

            # primer: hoist the ~1.3us ACT table load to t=0
            dm = sm.tile([P, 1], fp32)
            dmo = sm.tile([P, 1], fp32)
            nc.vector.memset(dm[:], 0.0)
            nc.scalar.activation(dmo[:], dm[:], AF.Exp)

            def red(acc_slot, src):
                # free-axis sum at 4x rate: ts (x*1), reduce-add seeded 0
                scr = prp.tile([P, C], bf16, tag="red")
                nc.vector.tensor_scalar(
                    scr[:], src, 1.0, 0.0, op0=ALU.mult, op1=ALU.add,
                    accum_out=acc_slot,
                )

            def red_fold(acc_slot, src, bout):
                # bout = src + 1 (=B); reduce-add seeded 0: accum = E + C
                nc.vector.tensor_scalar(
                    bout, src, 1.0, 0.0, op0=ALU.add, op1=ALU.add,
                    accum_out=acc_slot,
                )

            # all u vectors in one tile so sg passes can merge via strided APs
            U = sm.tile([P, 8 * C], bf16)

            # ---------------- DMA streams (two queues) ----------------
            xts = []
            for t in range(T):
                r0, r1 = t * P, (t + 1) * P
                if t == 0:
                    xt0 = xp.tile([P, 2 * C], fp32, tag="x0")
                    nc.sync.dma_start(xt0[:, 0:H], x1d[r0:r1, 0:H])
                    nc.sync.dma_start(xt0[:, H:C], x1d[r0:r1, H:C])
                    nc.gpsimd.dma_start(xt0[:, C : 2 * C], x2d[r0:r1, :])
                    xts.append((xt0[:, 0:C], xt0[:, C : 2 * C], xt0))
                else:
                    xt = xp.tile([P, 2 * C], fp32, tag="x")
                    nc.sync.dma_start(xt[:, 0:C], x1d[r0:r1, :])
                    q2 = nc.gpsimd if t == 2 else nc.sync
                    q2.dma_start(xt[:, C : 2 * C], x2d[r0:r1, :])
                    xts.append((xt[:, 0:C], xt[:, C : 2 * C], xt))

            # ---------------- main per-tile streams ----------------
            uts = []
            for t in range(T):
                x1s, x2s, xfull = xts[t]
                if t < 2:
                    At = ap_.tile([P, 2 * C], bf16, tag="A")
                else:
                    At = ap_.tile([P, 2 * C], bf16, tag="A2")
                # ---- ACT: exp (tile 0: first x1 half alone, then the rest
                # as one 1.5C instruction once both queues have landed) ----
                if t == 0:
                    nc.scalar.activation(At[:, 0:H], x1s[:, 0:H], AF.Exp)
                    nc.scalar.activation(
                        At[:, H : 2 * C], xfull[:, H : 2 * C], AF.Exp
                    )
                else:
                    nc.scalar.activation(At[:], xfull[:], AF.Exp)

                # ---- Pool: G product mults ----
                pg1 = prp.tile([P, C], bf16, tag="pg1")
                nc.gpsimd.tensor_tensor(pg1[:], At[:, 0:C], x2s, op=ALU.mult)
                pg2 = prp.tile([P, C], bf16, tag="pg2")
                nc.gpsimd.tensor_tensor(
                    pg2[:], At[:, C : 2 * C], x1s, op=ALU.mult
                )

                # ---- DVE: E sums (B folded), recips, G reduces ----
                B1t = brp.tile([P, C], bf16, tag="B1")
                red_fold(outt[:, t : t + 1], At[:, 0:C], B1t[:])
                B2t = brp.tile([P, C], bf16, tag="B2")
                red_fold(outt[:, T + t : T + t + 1], At[:, C : 2 * C], B2t[:])
                R2t = brp.tile([P, C], bf16, tag="R2")
                with nc.allow_low_precision("r feeds bf16 products"):
                    nc.vector.reciprocal(R2t[:], B2t[:])
                    if t == FOLD:
                        R1t = brp.tile([P, C], bf16, tag="R1")
                        nc.vector.reciprocal(R1t[:], B1t[:])
                red(outt[:, 2 * T + t : 2 * T + t + 1], pg1[:])
                red(outt[:, 3 * T + t : 3 * T + t + 1], pg2[:])

                # ---- ACT: softplus; Pool: u; DVE: fused M ----
                LLpt = llpp.tile([P, 2 * C], fp32, tag="llp")
                nc.scalar.activation(LLpt[:], At[:], AF.Ln, bias=1.0)
                ut = U[:, 2 * C * t : 2 * C * (t + 1)]
                nc.gpsimd.tensor_sub(ut[:, 0:C], x1s, LLpt[:, 0:C])
                nc.gpsimd.tensor_sub(ut[:, C : 2 * C], x2s, LLpt[:, C : 2 * C])
                uts.append(ut)
                scm = prp.tile([P, C], bf16, tag="scm")
                nc.vector.scalar_tensor_tensor(
                    scm[:], R2t[:], 1.0, ut[:, 0:C],
                    op0=ALU.subtract, op1=ALU.mult,
                    accum_out=outt[:, 5 * T + t : 5 * T + t + 1],
                )
                if t == FOLD:
                    # M12 folded: sum (r1-1)*u2 = -M12
                    scm2 = prp.tile([P, C], bf16, tag="scm2")
                    nc.vector.scalar_tensor_tensor(
                        scm2[:], R1t[:], 1.0, ut[:, C : 2 * C],
                        op0=ALU.subtract, op1=ALU.mult,
                        accum_out=outt[:, 4 * T + t : 4 * T + t + 1],
                    )

            # ---------------- late sg chains (M12 of SG tiles) ----------------
            # sg0 + sg2 as ONE strided-AP instruction over the shared u tile
            sg02 = sgp.tile([P, 2 * C], bf16, tag="sg02")
            u02 = U[:].rearrange("p (t c) -> p t c", c=C)[:, 0:5:4, :]
            s02 = sg02[:].rearrange("p (t c) -> p t c", c=C)
            nc.scalar.activation(s02, u02, AF.Exp)
            for i, t in enumerate((0, 2)):
                pm = prp.tile([P, C], bf16, tag="pm")
                nc.gpsimd.tensor_tensor(
                    pm[:], sg02[:, i * C : (i + 1) * C],
                    uts[t][:, C : 2 * C], op=ALU.mult,
                )
                red(outt[:, 4 * T + t : 4 * T + t + 1], pm[:])
            sgt = sgp.tile([P, C], bf16, tag="sg")
            nc.scalar.activation(sgt[:], uts[LAST][:, 0:C], AF.Exp)
            pm3 = prp.tile([P, C], bf16, tag="pm")
            nc.gpsimd.tensor_tensor(
                pm3[:], sgt[:], uts[LAST][:, C : 2 * C], op=ALU.mult
            )
            red(outt[:, 4 * T + LAST : 4 * T + LAST + 1], pm3[:])

            nc.sync.dma_start(outd, outt[:])

    nc.compile()
    return nc


def _get_nc():
    if "nc" not in _cache:
        _cache["nc"] = _build()
    return _cache["nc"]


def kernel(out1, out2, para, target, pos_idx):
    from concourse.bass_utils import run_bass_kernel_spmd

    nc = _get_nc()

    out1 = np.ascontiguousarray(out1, dtype=np.float32)
    out2 = np.ascontiguousarray(out2, dtype=np.float32)
    idx = pos_idx.astype(np.int64)
    g1 = np.take_along_axis(out1, idx, axis=1).astype(np.float64)  # [B, K]
    g2 = np.take_along_axis(out2, idx, axis=1).astype(np.float64)

    in_maps = [
        {
            "x1": out1[c * RPC : (c + 1) * RPC],
            "x2": out2[c * RPC : (c + 1) * RPC],
        }
        for c in range(NCORES)
    ]
    res = run_bass_kernel_spmd(nc, in_maps, core_ids=list(range(NCORES)))
    parts = np.stack([r["out"] for r in res.results])  # [NCORES, P, 6T]

    # unpack: col q*T+t of row p is global row c*RPC + t*P + p
    main = parts.reshape(NCORES, P, 6, T)
    q = main.transpose(0, 3, 1, 2).reshape(B, 6).astype(np.float64)
    E1, E2, G12, G21, M12, M21n = (q[:, i] for i in range(6))
    E1 = E1 - C          # B-fold adds C to the E accumulators
    E2 = E2 - C
    # FOLD tile: M12 accumulated as -M12
    for c in range(NCORES):
        sl = slice(c * RPC + FOLD * P, c * RPC + (FOLD + 1) * P)
        M12[sl] = -M12[sl]
    M21 = -M21n

    # host finale in fp64 (tiny [B,K] math)
    a1 = np.exp(g1)
    a2 = np.exp(g2)
    D1 = (E1 - a1.sum(1))[:, None] + a1
    D2 = (E2 - a2.sum(1))[:, None] + a2
    P12 = (a1 * g2).sum(1)
    P21 = (a2 * g1).sum(1)
    row_single = (
        np.log(D1).sum(1) + np.log(D2).sum(1)
        - (G12 - P12) * (1.0 / D1).sum(1) - (a1 * g2 / D1).sum(1)
        - (G21 - P21) * (1.0 / D2).sum(1) - (a2 * g1 / D2).sum(1)
    )
    single = row_single.sum() / (B * K)
    multi = -(M12.sum() + M21.sum()) / B
    p = float(np.asarray(para))
    return np.asarray(p * multi + (1.0 - p) * single, dtype=np.float32)
